# revision 8
# baseline (speedup 1.0000x reference)
"""Trainium2 Bass kernel for nn_CrossAttentionInjector.

Data-parallel over batch: one sample per NeuronCore (B=8 on 8 cores).
Per-core pipeline (all layouts transposed so contractions sit on partitions):
  qT = Wq @ h           (KV on partitions, S free)        f32r matmuls
  KT = Wk @ cond^T      (KV on partitions, N free)
  V  = cond @ Wv^T      (N on partitions, KV free, per-head 65-stride with
                         an appended ones column -> softmax denominators fall
                         out of the attention matmul for free)
  selector: centrality_i = phat_i . (sum_j phat_j)  (rank-equivalent to the
            reference's Smat row-sums); top-k mask via rank counting; mask
            fused into the exp() bias (per-partition, n on partitions)
  scoresT = KT_h^T-slices @ qT_h  (n on partitions, q free), exp on ScalarE
  attn@V with the ones column -> (65, q) PSUM, row 64 = denominator
  divide via reciprocal_approx + gpsimd partition_broadcast, out-proj, +bo
"""

import numpy as np

B, C, H, W = 8, 256, 32, 32
N = 512
COND = 512
KVD = 512
RD = 64
NH = 8
S = 1024
NEGB = 30000.0
N_CORES = 8

_cache = {}


def _build(stage=5):
    import concourse.tile as tile
    import concourse.mybir as mybir
    from concourse import bacc
    import contextlib

    f32 = mybir.dt.float32
    f32r = mybir.dt.float32r
    A = mybir.AluOpType
    AF = mybir.ActivationFunctionType

    nc = bacc.Bacc("TRN2", target_bir_lowering=False, debug=False)

    hS = nc.dram_tensor("hS", [C, S], f32r, kind="ExternalInput").ap()
    condT = nc.dram_tensor("condT", [COND, N], f32r, kind="ExternalInput").ap()
    maskc = nc.dram_tensor("maskc", [128, 4], f32, kind="ExternalInput").ap()
    WqT = nc.dram_tensor("WqT", [C, KVD], f32r, kind="ExternalInput").ap()
    WkT = nc.dram_tensor("WkT", [COND, KVD], f32r, kind="ExternalInput").ap()
    WvT = nc.dram_tensor("WvT", [COND, KVD], f32r, kind="ExternalInput").ap()
    WoT = nc.dram_tensor("WoT", [KVD, C], f32r, kind="ExternalInput").ap()
    WrkT = nc.dram_tensor("WrkT", [COND, RD], f32, kind="ExternalInput").ap()
    bqc = nc.dram_tensor("bqc", [128, 4], f32, kind="ExternalInput").ap()
    bkc = nc.dram_tensor("bkc", [128, 4], f32, kind="ExternalInput").ap()
    bvB = nc.dram_tensor("bvB", [128, KVD], f32, kind="ExternalInput").ap()
    brkB = nc.dram_tensor("brkB", [128, RD], f32, kind="ExternalInput").ap()
    boc = nc.dram_tensor("boc", [128, 2], f32, kind="ExternalInput").ap()
    onesc = nc.dram_tensor("onesc", [128, 1], f32, kind="ExternalInput").ap()
    onesr = nc.dram_tensor("onesr", [128, 1], f32r, kind="ExternalInput").ap()
    y = nc.dram_tensor("y", [C, S], f32, kind="ExternalOutput").ap()

    with tile.TileContext(nc) as tc, contextlib.ExitStack() as ctx:
        cons = ctx.enter_context(tc.tile_pool(name="cons", bufs=1))
        work = ctx.enter_context(tc.tile_pool(name="work", bufs=1))
        ppool = ctx.enter_context(tc.tile_pool(name="ppool", bufs=8))
        rbp = ctx.enter_context(tc.tile_pool(name="rbp", bufs=4))
        psS = ctx.enter_context(tc.tile_pool(name="psS", bufs=2, space="PSUM"))
        psM = ctx.enter_context(tc.tile_pool(name="psM", bufs=2, space="PSUM"))
        psA = ctx.enter_context(tc.tile_pool(name="psA", bufs=2, space="PSUM"))

        # ---------------- input DMAs ----------------
        h_t = [cons.tile([128, S], f32r, tag=f"h{i}", name=f"h{i}") for i in range(2)]
        for i in range(2):
            nc.sync.dma_start(h_t[i][:], hS[128 * i:128 * (i + 1), :])
        ct_t = [cons.tile([128, N], f32r, tag=f"ct{i}", name=f"ct{i}") for i in range(4)]
        for i in range(4):
            nc.sync.dma_start(ct_t[i][:], condT[128 * i:128 * (i + 1), :])
        wq_t = [cons.tile([128, KVD], f32r, tag=f"wq{i}", name=f"wq{i}") for i in range(2)]
        for i in range(2):
            nc.sync.dma_start(wq_t[i][:], WqT[128 * i:128 * (i + 1), :])
        wk_t = [cons.tile([128, KVD], f32r, tag=f"wk{i}", name=f"wk{i}") for i in range(4)]
        wv_t = [cons.tile([128, KVD], f32r, tag=f"wv{i}", name=f"wv{i}") for i in range(4)]
        wr_t = [cons.tile([128, RD], f32, tag=f"wr{i}", name=f"wr{i}") for i in range(4)]
        for i in range(4):
            nc.sync.dma_start(wk_t[i][:], WkT[128 * i:128 * (i + 1), :])
            nc.sync.dma_start(wv_t[i][:], WvT[128 * i:128 * (i + 1), :])
            nc.sync.dma_start(wr_t[i][:], WrkT[128 * i:128 * (i + 1), :])
        wo_t = [cons.tile([128, C], f32r, tag=f"wo{i}", name=f"wo{i}") for i in range(4)]
        for i in range(4):
            nc.sync.dma_start(wo_t[i][:], WoT[128 * i:128 * (i + 1), :])
        maskc_t = cons.tile([128, 4], f32, tag="maskc")
        nc.sync.dma_start(maskc_t[:], maskc)
        bqc_t = cons.tile([128, 4], f32, tag="bqc")
        nc.sync.dma_start(bqc_t[:], bqc)
        bkc_t = cons.tile([128, 4], f32, tag="bkc")
        nc.sync.dma_start(bkc_t[:], bkc)
        bvB_t = cons.tile([128, KVD], f32, tag="bvB")
        nc.sync.dma_start(bvB_t[:], bvB)
        brkB_t = cons.tile([128, RD], f32, tag="brkB")
        nc.sync.dma_start(brkB_t[:], brkB)
        boc_t = cons.tile([128, 2], f32, tag="boc")
        nc.sync.dma_start(boc_t[:], boc)
        onesc_t = cons.tile([128, 1], f32, tag="onesc")
        nc.sync.dma_start(onesc_t[:], onesc)
        onesr_t = cons.tile([128, 1], f32r, tag="onesr")
        nc.sync.dma_start(onesr_t[:], onesr)

        # ---------------- selector (full fp32 path) ----------------
        c4 = work.tile([128, 4], f32, tag="c4")
        rank4 = work.tile([128, 4], f32, tag="rank4")
        biasb = work.tile([128, 4], f32, tag="biasb")
        ph_l = []
        for i in range(4):
            pp = psM.tile([128, RD], f32, tag="psM")
            for cc in range(4):
                nc.tensor.matmul(pp[:], ct_t[cc][:, 128 * i:128 * (i + 1)].bitcast(f32),
                                 wr_t[cc][:], start=(cc == 0), stop=(cc == 3))
            Pn = work.tile([128, RD], f32, tag=f"Pn{i}", name=f"Pn{i}")
            nc.vector.tensor_tensor(Pn[:], pp[:], brkB_t[:], op=A.add)
            tmp64 = work.tile([128, RD], f32, tag=f"tmp64_{i}", name=f"tmp64_{i}")
            sq = work.tile([128, 1], f32, tag=f"sq{i}", name=f"sq{i}")
            nc.vector.tensor_tensor(tmp64[:], Pn[:], Pn[:], op=A.mult)
            nc.vector.reduce_sum(sq[:], tmp64[:], axis=mybir.AxisListType.X)
            lns = work.tile([128, 1], f32, tag=f"lns{i}", name=f"lns{i}")
            nc.scalar.activation(lns[:], sq[:], AF.Ln)
            rn = work.tile([128, 1], f32, tag=f"rn{i}", name=f"rn{i}")
            nc.scalar.activation(rn[:], lns[:], AF.Exp, scale=-0.5)
            ph = work.tile([128, RD], f32, tag=f"ph{i}", name=f"ph{i}")
            nc.vector.tensor_scalar(ph[:], Pn[:], rn[:, 0:1], None, op0=A.mult)
            ph_l.append(ph)
        sps = psM.tile([1, RD], f32, tag="psM")
        for i in range(4):
            nc.tensor.matmul(sps[:], onesc_t[:], ph_l[i][:], start=(i == 0), stop=(i == 3))
        s_row = work.tile([1, RD], f32, tag="s_row")
        nc.vector.tensor_copy(s_row[:], sps[:])
        sB = work.tile([128, RD], f32, tag="sB")
        nc.gpsimd.partition_broadcast(sB[:], s_row[:])
        for i in range(4):
            tmp64b = work.tile([128, RD], f32, tag=f"tmp64b_{i}", name=f"tmp64b_{i}")
            nc.vector.tensor_tensor(tmp64b[:], ph_l[i][:], sB[:], op=A.mult)
            nc.vector.reduce_sum(c4[:, i:i + 1], tmp64b[:], axis=mybir.AxisListType.X)
        cB = work.tile([128, N], f32, tag="cB")
        for j in range(4):
            crow = work.tile([1, 128], f32, tag=f"crow{j}", name=f"crow{j}")
            nc.sync.dma_start(crow[:], c4[:, j:j + 1])
            nc.gpsimd.partition_broadcast(cB[:, 128 * j:128 * (j + 1)], crow[:])
        cmpd = work.tile([128, N], f32, tag="cmpd")
        for i in range(4):
            nc.vector.tensor_scalar(cmpd[:], cB[:], c4[:, i:i + 1], 0.0,
                                    op0=A.is_gt, op1=A.add,
                                    accum_out=rank4[:, i:i + 1])
        selm = work.tile([128, 4], f32, tag="selm")
        nc.vector.tensor_scalar(selm[:], rank4[:], 306.5, None, op0=A.is_lt)
        allowed4 = work.tile([128, 4], f32, tag="allowed4")
        nc.vector.tensor_tensor(allowed4[:], selm[:], maskc_t[:], op=A.mult)
        nc.vector.tensor_scalar(biasb[:], allowed4[:], NEGB, NEGB,
                                op0=A.mult, op1=A.subtract)
        if stage < 2:
            nc.sync.dma_start(y[0:128, 0:4], biasb[:])
        # ---------------- projections (f32r) ----------------
        if stage < 2:
            nc.compile_marker = None
        kt_t = [work.tile([128, N], f32r, tag=f"kt{i}", name=f"kt{i}") for i in range(4)]
        for kv in range(4):
            ps = psM.tile([128, N], f32, tag="psM")
            for cc in range(4):
                nc.tensor.matmul(ps[:], wk_t[cc][:, 128 * kv:128 * (kv + 1)],
                                 ct_t[cc][:], start=(cc == 0), stop=(cc == 3))
            nc.vector.tensor_scalar(kt_t[kv][:], ps[:], bkc_t[:, kv:kv + 1], None, op0=A.add)

        v520 = [work.tile([128, 520], f32r, tag=f"v520_{i}", name=f"v520_{i}") for i in range(4)]
        for nn_ in range(4):
            for hh in range(NH):
                nc.vector.tensor_copy(v520[nn_][:, 65 * hh + 64:65 * hh + 65], onesr_t[:])
            ps = psM.tile([128, KVD], f32, tag="psM")
            for cc in range(4):
                nc.tensor.matmul(ps[:], ct_t[cc][:, 128 * nn_:128 * (nn_ + 1)],
                                 wv_t[cc][:], start=(cc == 0), stop=(cc == 3))
            vview = v520[nn_][:].rearrange("p (h c) -> p h c", c=65)[:, :, 0:64]
            nc.vector.tensor_tensor(vview,
                                    ps[:].rearrange("p (h c) -> p h c", c=64),
                                    bvB_t[:].rearrange("p (h c) -> p h c", c=64),
                                    op=A.add)

        qt_t = [work.tile([128, S], f32r, tag=f"qt{i}", name=f"qt{i}") for i in range(4)]
        for kv in range(4):
            for sc in range(2):
                ps = psM.tile([128, 512], f32, tag="psM")
                for cc in range(2):
                    nc.tensor.matmul(ps[:], wq_t[cc][:, 128 * kv:128 * (kv + 1)],
                                     h_t[cc][:, 512 * sc:512 * (sc + 1)],
                                     start=(cc == 0), stop=(cc == 1))
                nc.vector.tensor_scalar(qt_t[kv][:, 512 * sc:512 * (sc + 1)], ps[:],
                                        bqc_t[:, kv:kv + 1], None, op0=A.add)
        if stage < 3:
            nc.sync.dma_start(y[0:128, :], qt_t[0][:].bitcast(f32))
        # ---------------- attention ----------------
        att_t = [work.tile([128, S], f32r, tag=f"att{i}", name=f"att{i}") for i in range(4)]
        dstage = work.tile([16, 512], f32, tag="dstage")
        drecip = work.tile([16, 512], f32, tag="drecip")
        dscr = work.tile([16, 512], f32, tag="dscr")
        o65_l = {}
        for h in range(NH):
            i2 = h // 2
            po = (h % 2) * 64
            plist = []
            for nn_ in range(4):
                sps_ = psS.tile([128, 1024], f32, tag="psS")
                for qc in range(2):
                    nc.tensor.matmul(sps_[:, 512 * qc:512 * (qc + 1)],
                                     kt_t[i2][po:po + 64, 128 * nn_:128 * (nn_ + 1)],
                                     qt_t[i2][po:po + 64, 512 * qc:512 * (qc + 1)],
                                     start=True, stop=True)
                p_t = ppool.tile([128, 1024], f32r, tag="p", name=f"p_{h}_{nn_}")
                nc.scalar.activation(p_t[:], sps_[:], AF.Exp,
                                     bias=biasb[:, nn_:nn_ + 1], scale=0.125)
                plist.append(p_t)
            if stage < 4:
                continue
            for qc in range(2):
                aug = psA.tile([65, 512], f32, tag="psA")
                for nn_ in range(4):
                    nc.tensor.matmul(aug[:], v520[nn_][:, 65 * h:65 * h + 65],
                                     plist[nn_][:, 512 * qc:512 * (qc + 1)],
                                     start=(nn_ == 0), stop=(nn_ == 3))
                k = 2 * h + qc
                o65 = work.tile([65, 512], f32, tag=f"o65_{k}", name=f"o65_{k}")
                nc.vector.tensor_copy(o65[:], aug[:])
                nc.sync.dma_start(dstage[k:k + 1, :], o65[64:65, :])
                o65_l[k] = o65
        if stage >= 4:
            nc.vector.reciprocal_approx_accurate(drecip[:], dstage[:], dscr[:])
        if stage >= 5:
            for h in range(NH):
                i2 = h // 2
                po = (h % 2) * 64
                for qc in range(2):
                    k = 2 * h + qc
                    rrow = rbp.tile([1, 512], f32, tag="rrow", name=f"rrow{k}")
                    nc.sync.dma_start(rrow[:], drecip[k:k + 1, :])
                    rB = rbp.tile([64, 512], f32, tag="rB", name=f"rB{k}")
                    nc.gpsimd.partition_broadcast(rB[:], rrow[:])
                    nc.vector.tensor_tensor(att_t[i2][po:po + 64, 512 * qc:512 * (qc + 1)],
                                            o65_l[k][0:64, :], rB[:], op=A.mult)

            # ---------------- output projection ----------------
            outF = [work.tile([128, S], f32, tag=f"outF{i}", name=f"outF{i}") for i in range(2)]
            for ccn in range(2):
                for sc in range(2):
                    ps = psM.tile([128, 512], f32, tag="psM")
                    for kv in range(4):
                        nc.tensor.matmul(ps[:], wo_t[kv][:, 128 * ccn:128 * (ccn + 1)],
                                         att_t[kv][:, 512 * sc:512 * (sc + 1)],
                                         start=(kv == 0), stop=(kv == 3))
                    nc.scalar.add(outF[ccn][:, 512 * sc:512 * (sc + 1)], ps[:],
                                  boc_t[:, ccn:ccn + 1])
            for ccn in range(2):
                nc.sync.dma_start(y[128 * ccn:128 * (ccn + 1), :], outF[ccn][:])
        elif stage == 4:
            nc.sync.dma_start(y[0:16, 0:512], drecip[:])
        elif stage == 3:
            nc.sync.dma_start(y[0:128, 0:512], plist[3][:, 0:512].bitcast(f32))

    nc.compile()
    return nc


def _get_nc(stage=5):
    key = f"nc{stage}"
    if key not in _cache:
        _cache[key] = _build(stage)
    return _cache[key]


def make_in_maps(**inputs):
    h = np.asarray(inputs["h"], np.float32)
    cond = np.asarray(inputs["cond_feats"], np.float32)
    cmask = np.asarray(inputs["cond_mask"])
    f = np.float32
    shared = {
        "WqT": np.ascontiguousarray(np.asarray(inputs["Wq"], f).T),
        "WkT": np.ascontiguousarray(np.asarray(inputs["Wk"], f).T),
        "WvT": np.ascontiguousarray(np.asarray(inputs["Wv"], f).T),
        "WoT": np.ascontiguousarray(np.asarray(inputs["Wo"], f).T),
        "WrkT": np.ascontiguousarray(np.asarray(inputs["Wrk"], f).T),
        "bqc": np.ascontiguousarray(np.asarray(inputs["bq"], f).reshape(4, 128).T),
        "bkc": np.ascontiguousarray(np.asarray(inputs["bk"], f).reshape(4, 128).T),
        "bvB": np.ascontiguousarray(np.broadcast_to(np.asarray(inputs["bv"], f), (128, KVD))),
        "brkB": np.ascontiguousarray(np.broadcast_to(np.asarray(inputs["brk"], f), (128, RD))),
        "boc": np.ascontiguousarray(np.asarray(inputs["bo"], f).reshape(2, 128).T),
        "onesc": np.ones((128, 1), f),
        "onesr": np.ones((128, 1), f),
    }
    in_maps = []
    for b in range(B):
        m = dict(shared)
        m["hS"] = np.ascontiguousarray(h[b].reshape(C, S))
        m["condT"] = np.ascontiguousarray(cond[b].T)
        m["maskc"] = np.ascontiguousarray(cmask[b].astype(f).reshape(4, 128).T)
        in_maps.append(m)
    return in_maps


def kernel(**inputs):
    from concourse.bass_utils import run_bass_kernel_spmd
    nc = _get_nc()
    in_maps = make_in_maps(**inputs)
    res = run_bass_kernel_spmd(nc, in_maps, core_ids=list(range(N_CORES)))
    return np.stack([res.results[b]["y"].reshape(C, H, W) for b in range(B)])


# revision 12
# speedup vs baseline: 1.0973x; 1.0973x over previous
"""Trainium2 Bass kernel for nn_CrossAttentionInjector.

Data-parallel over batch: one sample per NeuronCore (B=8 on 8 cores).
Per-core pipeline (all layouts transposed so contractions sit on partitions):
  qT = Wq @ h           (KV on partitions, S free)        f32r matmuls
  KT = Wk @ cond^T      (KV on partitions, N free)
  V  = cond @ Wv^T      (N on partitions, KV free, per-head 65-stride with
                         an appended ones column -> softmax denominators fall
                         out of the attention matmul for free)
  selector: centrality_i = phat_i . (sum_j phat_j)  (rank-equivalent to the
            reference's Smat row-sums); top-k mask via rank counting; mask
            fused into the exp() bias (per-partition, n on partitions)
  scoresT = KT_h^T-slices @ qT_h  (n on partitions, q free), exp on ScalarE
  attn@V with the ones column -> (65, q) PSUM, row 64 = denominator
  divide via reciprocal_approx + f32r ones-outer-product broadcast matmul,
  out-proj with +bo fused into the PSUM->SBUF copy on ScalarE
"""

import numpy as np

B, C, H, W = 8, 256, 32, 32
N = 512
COND = 512
KVD = 512
RD = 64
NH = 8
S = 1024
NEGB = 30000.0
N_CORES = 8

_cache = {}


def _build():
    import concourse.tile as tile
    import concourse.mybir as mybir
    from concourse import bacc
    import contextlib

    f32 = mybir.dt.float32
    f32r = mybir.dt.float32r
    A = mybir.AluOpType
    AF = mybir.ActivationFunctionType

    nc = bacc.Bacc("TRN2", target_bir_lowering=False, debug=False)

    hS = nc.dram_tensor("hS", [C, S], f32r, kind="ExternalInput").ap()
    condT = nc.dram_tensor("condT", [COND, N], f32r, kind="ExternalInput").ap()
    maskc = nc.dram_tensor("maskc", [128, 4], f32, kind="ExternalInput").ap()
    WqT = nc.dram_tensor("WqT", [C, KVD], f32r, kind="ExternalInput").ap()
    WkT = nc.dram_tensor("WkT", [COND, KVD], f32r, kind="ExternalInput").ap()
    WvT = nc.dram_tensor("WvT", [COND, KVD], f32r, kind="ExternalInput").ap()
    WoT = nc.dram_tensor("WoT", [KVD, C], f32r, kind="ExternalInput").ap()
    WrkT = nc.dram_tensor("WrkT", [COND, RD], f32, kind="ExternalInput").ap()
    bqc = nc.dram_tensor("bqc", [128, 4], f32, kind="ExternalInput").ap()
    bkc = nc.dram_tensor("bkc", [128, 4], f32, kind="ExternalInput").ap()
    bvB = nc.dram_tensor("bvB", [128, KVD], f32, kind="ExternalInput").ap()
    brkB = nc.dram_tensor("brkB", [128, RD], f32, kind="ExternalInput").ap()
    boc = nc.dram_tensor("boc", [128, 2], f32, kind="ExternalInput").ap()
    onesc = nc.dram_tensor("onesc", [128, 1], f32, kind="ExternalInput").ap()
    onesr = nc.dram_tensor("onesr", [128, 1], f32r, kind="ExternalInput").ap()
    onesw = nc.dram_tensor("onesw", [1, 64], f32r, kind="ExternalInput").ap()
    y = nc.dram_tensor("y", [C, S], f32, kind="ExternalOutput").ap()

    with tile.TileContext(nc) as tc, contextlib.ExitStack() as ctx:
        cons = ctx.enter_context(tc.tile_pool(name="cons", bufs=1))
        work = ctx.enter_context(tc.tile_pool(name="work", bufs=1))
        ppool = ctx.enter_context(tc.tile_pool(name="ppool", bufs=8))

        # ---------------- input DMAs (selector deps first) ----------------
        ct_t = [cons.tile([128, N], f32r, tag=f"ct{i}", name=f"ct{i}") for i in range(4)]
        for i in range(4):
            nc.sync.dma_start(ct_t[i][:], condT[128 * i:128 * (i + 1), :])
        wr_t = [cons.tile([128, RD], f32, tag=f"wr{i}", name=f"wr{i}") for i in range(4)]
        for i in range(4):
            nc.sync.dma_start(wr_t[i][:], WrkT[128 * i:128 * (i + 1), :])
        brkB_t = cons.tile([128, RD], f32, tag="brkB")
        nc.sync.dma_start(brkB_t[:], brkB)
        maskc_t = cons.tile([128, 4], f32, tag="maskc")
        nc.sync.dma_start(maskc_t[:], maskc)
        onesc_t = cons.tile([128, 1], f32, tag="onesc")
        nc.sync.dma_start(onesc_t[:], onesc)
        onesr_t = cons.tile([128, 1], f32r, tag="onesr")
        nc.sync.dma_start(onesr_t[:], onesr)
        onesw_t = cons.tile([1, 64], f32r, tag="onesw")
        nc.sync.dma_start(onesw_t[:], onesw)
        wk_t = [cons.tile([128, KVD], f32r, tag=f"wk{i}", name=f"wk{i}") for i in range(4)]
        wv_t = [cons.tile([128, KVD], f32r, tag=f"wv{i}", name=f"wv{i}") for i in range(4)]
        for i in range(4):
            nc.sync.dma_start(wk_t[i][:], WkT[128 * i:128 * (i + 1), :])
            nc.sync.dma_start(wv_t[i][:], WvT[128 * i:128 * (i + 1), :])
        wq_t = [cons.tile([128, KVD], f32r, tag=f"wq{i}", name=f"wq{i}") for i in range(2)]
        for i in range(2):
            nc.sync.dma_start(wq_t[i][:], WqT[128 * i:128 * (i + 1), :])
        h_t = [cons.tile([128, S], f32r, tag=f"h{i}", name=f"h{i}") for i in range(2)]
        for i in range(2):
            nc.sync.dma_start(h_t[i][:], hS[128 * i:128 * (i + 1), :])
        wo_t = [cons.tile([128, C], f32r, tag=f"wo{i}", name=f"wo{i}") for i in range(4)]
        for i in range(4):
            nc.sync.dma_start(wo_t[i][:], WoT[128 * i:128 * (i + 1), :])
        bqc_t = cons.tile([128, 4], f32, tag="bqc")
        nc.sync.dma_start(bqc_t[:], bqc)
        bkc_t = cons.tile([128, 4], f32, tag="bkc")
        nc.sync.dma_start(bkc_t[:], bkc)
        bvB_t = cons.tile([128, KVD], f32, tag="bvB")
        nc.sync.dma_start(bvB_t[:], bvB)
        boc_t = cons.tile([128, 2], f32, tag="boc")
        nc.sync.dma_start(boc_t[:], boc)

        with tc.tile_pool(name="psSel", bufs=2, space="PSUM") as psSel, \
             tc.tile_pool(name="psM", bufs=3, space="PSUM") as psM:
            # ---------------- selector (full fp32 path) ----------------
            c4 = work.tile([128, 4], f32, tag="c4")
            rank4 = work.tile([128, 4], f32, tag="rank4")
            biasb = work.tile([128, 4], f32, tag="biasb")
            ph_l = []
            for i in range(4):
                pp = psSel.tile([128, RD], f32, tag="psSel")
                for cc in range(4):
                    nc.tensor.matmul(pp[:], ct_t[cc][:, 128 * i:128 * (i + 1)].bitcast(f32),
                                     wr_t[cc][:], start=(cc == 0), stop=(cc == 3))
                Pn = work.tile([128, RD], f32, tag=f"Pn{i}", name=f"Pn{i}")
                nc.vector.tensor_tensor(Pn[:], pp[:], brkB_t[:], op=A.add)
                tmp64 = work.tile([128, RD], f32, tag=f"tmp64_{i}", name=f"tmp64_{i}")
                sq = work.tile([128, 1], f32, tag=f"sq{i}", name=f"sq{i}")
                nc.vector.tensor_tensor(tmp64[:], Pn[:], Pn[:], op=A.mult)
                nc.vector.reduce_sum(sq[:], tmp64[:], axis=mybir.AxisListType.X)
                lns = work.tile([128, 1], f32, tag=f"lns{i}", name=f"lns{i}")
                nc.scalar.activation(lns[:], sq[:], AF.Ln)
                rn = work.tile([128, 1], f32, tag=f"rn{i}", name=f"rn{i}")
                nc.scalar.activation(rn[:], lns[:], AF.Exp, scale=-0.5)
                ph = work.tile([128, RD], f32, tag=f"ph{i}", name=f"ph{i}")
                nc.vector.tensor_scalar(ph[:], Pn[:], rn[:, 0:1], None, op0=A.mult)
                ph_l.append(ph)
            sps = psSel.tile([1, RD], f32, tag="psSel")
            for i in range(4):
                nc.tensor.matmul(sps[:], onesc_t[:], ph_l[i][:], start=(i == 0), stop=(i == 3))
            s_row = work.tile([1, RD], f32, tag="s_row")
            nc.vector.tensor_copy(s_row[:], sps[:])
            sB = work.tile([128, RD], f32, tag="sB")
            nc.gpsimd.partition_broadcast(sB[:], s_row[:])
            for i in range(4):
                tmp64b = work.tile([128, RD], f32, tag=f"tmp64b_{i}", name=f"tmp64b_{i}")
                nc.vector.tensor_tensor(tmp64b[:], ph_l[i][:], sB[:], op=A.mult)
                nc.vector.reduce_sum(c4[:, i:i + 1], tmp64b[:], axis=mybir.AxisListType.X)
            cB = work.tile([128, N], f32, tag="cB")
            for j in range(4):
                crow = work.tile([1, 128], f32, tag=f"crow{j}", name=f"crow{j}")
                nc.sync.dma_start(crow[:], c4[:, j:j + 1])
                nc.gpsimd.partition_broadcast(cB[:, 128 * j:128 * (j + 1)], crow[:])
            cmpd = work.tile([128, N], f32, tag="cmpd")
            for i in range(4):
                nc.vector.tensor_scalar(cmpd[:], cB[:], c4[:, i:i + 1], 0.0,
                                        op0=A.is_gt, op1=A.add,
                                        accum_out=rank4[:, i:i + 1])
            selm = work.tile([128, 4], f32, tag="selm")
            nc.vector.tensor_scalar(selm[:], rank4[:], 306.5, None, op0=A.is_lt)
            allowed4 = work.tile([128, 4], f32, tag="allowed4")
            nc.vector.tensor_tensor(allowed4[:], selm[:], maskc_t[:], op=A.mult)
            nc.vector.tensor_scalar(biasb[:], allowed4[:], NEGB, NEGB,
                                    op0=A.mult, op1=A.subtract)

            # ---------------- projections (f32r) ----------------
            kt_t = [work.tile([128, N], f32r, tag=f"kt{i}", name=f"kt{i}") for i in range(4)]
            for kv in range(4):
                ps = psM.tile([128, N], f32, tag="psM")
                for cc in range(4):
                    nc.tensor.matmul(ps[:], wk_t[cc][:, 128 * kv:128 * (kv + 1)],
                                     ct_t[cc][:], start=(cc == 0), stop=(cc == 3))
                nc.vector.tensor_scalar(kt_t[kv][:], ps[:], bkc_t[:, kv:kv + 1], None, op0=A.add)

            v520 = [work.tile([128, 520], f32r, tag=f"v520_{i}", name=f"v520_{i}") for i in range(4)]
            for nn_ in range(4):
                for hh in range(NH):
                    nc.vector.tensor_copy(v520[nn_][:, 65 * hh + 64:65 * hh + 65], onesr_t[:])
                ps = psM.tile([128, KVD], f32, tag="psM")
                for cc in range(4):
                    nc.tensor.matmul(ps[:], ct_t[cc][:, 128 * nn_:128 * (nn_ + 1)],
                                     wv_t[cc][:], start=(cc == 0), stop=(cc == 3))
                vview = v520[nn_][:].rearrange("p (h c) -> p h c", c=65)[:, :, 0:64]
                nc.vector.tensor_tensor(vview,
                                        ps[:].rearrange("p (h c) -> p h c", c=64),
                                        bvB_t[:].rearrange("p (h c) -> p h c", c=64),
                                        op=A.add)

            qt_t = [work.tile([128, S], f32r, tag=f"qt{i}", name=f"qt{i}") for i in range(4)]
            for kv in range(4):
                for sc in range(2):
                    ps = psM.tile([128, 512], f32, tag="psM")
                    for cc in range(2):
                        nc.tensor.matmul(ps[:], wq_t[cc][:, 128 * kv:128 * (kv + 1)],
                                         h_t[cc][:, 512 * sc:512 * (sc + 1)],
                                         start=(cc == 0), stop=(cc == 1))
                    nc.vector.tensor_scalar(qt_t[kv][:, 512 * sc:512 * (sc + 1)], ps[:],
                                            bqc_t[:, kv:kv + 1], None, op0=A.add)

        # ---------------- attention ----------------
        with tc.tile_pool(name="psS", bufs=2, space="PSUM") as psS, \
             tc.tile_pool(name="psA", bufs=2, space="PSUM") as psA, \
             tc.tile_pool(name="psB", bufs=2, space="PSUM") as psB:
            att_t = [work.tile([128, S], f32r, tag=f"att{i}", name=f"att{i}") for i in range(4)]
            o65_l = {}
            for g in range(2):  # head groups (0-3, 4-7): denom/recip per group
                dstage = work.tile([8, 512], f32, tag=f"dstage{g}", name=f"dstage{g}")
                drecip = work.tile([8, 512], f32, tag=f"drecip{g}", name=f"drecip{g}")
                drecR = work.tile([8, 512], f32r, tag=f"drecR{g}", name=f"drecR{g}")
                dscr = work.tile([8, 512], f32, tag=f"dscr{g}", name=f"dscr{g}")
                for h in range(4 * g, 4 * g + 4):
                    i2 = h // 2
                    po = (h % 2) * 64
                    plist = []
                    for nn_ in range(4):
                        sps_ = psS.tile([128, 1024], f32, tag="psS")
                        for qc in range(2):
                            nc.tensor.matmul(sps_[:, 512 * qc:512 * (qc + 1)],
                                             kt_t[i2][po:po + 64, 128 * nn_:128 * (nn_ + 1)],
                                             qt_t[i2][po:po + 64, 512 * qc:512 * (qc + 1)],
                                             start=True, stop=True)
                        p_t = ppool.tile([128, 1024], f32r, tag="p", name=f"p_{h}_{nn_}")
                        nc.scalar.activation(p_t[:], sps_[:], AF.Exp,
                                             bias=biasb[:, nn_:nn_ + 1], scale=0.125)
                        plist.append(p_t)
                    augs = [psA.tile([65, 512], f32, tag="psA", name=f"aug_{h}_{qc}")
                            for qc in range(2)]
                    for nn_ in range(4):
                        for qc in range(2):
                            nc.tensor.matmul(augs[qc][:], v520[nn_][:, 65 * h:65 * h + 65],
                                             plist[nn_][:, 512 * qc:512 * (qc + 1)],
                                             start=(nn_ == 0), stop=(nn_ == 3))
                    for qc in range(2):
                        k = 2 * h + qc
                        o65 = work.tile([65, 512], f32, tag=f"o65_{k % 8}", name=f"o65_{k}")
                        nc.vector.tensor_copy(o65[:], augs[qc][:])
                        nc.sync.dma_start(dstage[k - 8 * g:k - 8 * g + 1, :], o65[64:65, :])
                        o65_l[k] = o65
                nc.vector.reciprocal_approx_accurate(drecip[:], dstage[:], dscr[:])
                nc.vector.tensor_copy(drecR[:], drecip[:])
                for h in range(4 * g, 4 * g + 4):
                    i2 = h // 2
                    po = (h % 2) * 64
                    for qc in range(2):
                        k = 2 * h + qc
                        rrow = work.tile([1, 512], f32r, tag=f"rrow{k}", name=f"rrow{k}")
                        nc.sync.dma_start(rrow[:], drecR[k - 8 * g:k - 8 * g + 1, :])
                        rps = psB.tile([64, 512], f32, tag="psB")
                        nc.tensor.matmul(rps[:], onesw_t[:], rrow[:],
                                         start=True, stop=True)
                        nc.vector.tensor_tensor(att_t[i2][po:po + 64, 512 * qc:512 * (qc + 1)],
                                                o65_l[k][0:64, :], rps[:], op=A.mult)

            # ---------------- output projection ----------------
            outF = [work.tile([128, S], f32, tag=f"outF{i}", name=f"outF{i}") for i in range(2)]
            for ccn in range(2):
                for sc in range(2):
                    ps = psB.tile([128, 512], f32, tag="psB")
                    for kv in range(4):
                        nc.tensor.matmul(ps[:], wo_t[kv][:, 128 * ccn:128 * (ccn + 1)],
                                         att_t[kv][:, 512 * sc:512 * (sc + 1)],
                                         start=(kv == 0), stop=(kv == 3))
                    nc.scalar.add(outF[ccn][:, 512 * sc:512 * (sc + 1)], ps[:],
                                  boc_t[:, ccn:ccn + 1])
            for ccn in range(2):
                nc.sync.dma_start(y[128 * ccn:128 * (ccn + 1), :], outF[ccn][:])

    nc.compile()
    return nc


def _get_nc():
    if "nc" not in _cache:
        _cache["nc"] = _build()
    return _cache["nc"]


def make_in_maps(**inputs):
    h = np.asarray(inputs["h"], np.float32)
    cond = np.asarray(inputs["cond_feats"], np.float32)
    cmask = np.asarray(inputs["cond_mask"])
    f = np.float32
    shared = {
        "WqT": np.ascontiguousarray(np.asarray(inputs["Wq"], f).T),
        "WkT": np.ascontiguousarray(np.asarray(inputs["Wk"], f).T),
        "WvT": np.ascontiguousarray(np.asarray(inputs["Wv"], f).T),
        "WoT": np.ascontiguousarray(np.asarray(inputs["Wo"], f).T),
        "WrkT": np.ascontiguousarray(np.asarray(inputs["Wrk"], f).T),
        "bqc": np.ascontiguousarray(np.asarray(inputs["bq"], f).reshape(4, 128).T),
        "bkc": np.ascontiguousarray(np.asarray(inputs["bk"], f).reshape(4, 128).T),
        "bvB": np.ascontiguousarray(np.broadcast_to(np.asarray(inputs["bv"], f), (128, KVD))),
        "brkB": np.ascontiguousarray(np.broadcast_to(np.asarray(inputs["brk"], f), (128, RD))),
        "boc": np.ascontiguousarray(np.asarray(inputs["bo"], f).reshape(2, 128).T),
        "onesc": np.ones((128, 1), f),
        "onesr": np.ones((128, 1), f),
        "onesw": np.ones((1, 64), f),
    }
    in_maps = []
    for b in range(B):
        m = dict(shared)
        m["hS"] = np.ascontiguousarray(h[b].reshape(C, S))
        m["condT"] = np.ascontiguousarray(cond[b].T)
        m["maskc"] = np.ascontiguousarray(cmask[b].astype(f).reshape(4, 128).T)
        in_maps.append(m)
    return in_maps


def kernel(**inputs):
    from concourse.bass_utils import run_bass_kernel_spmd
    nc = _get_nc()
    in_maps = make_in_maps(**inputs)
    res = run_bass_kernel_spmd(nc, in_maps, core_ids=list(range(N_CORES)))
    return np.stack([res.results[b]["y"].reshape(C, H, W) for b in range(B)])


# revision 13
# speedup vs baseline: 1.3199x; 1.2029x over previous
"""Trainium2 Bass kernel for nn_CrossAttentionInjector.

Data-parallel over batch: one sample per NeuronCore (B=8 on 8 cores).
Per-core pipeline (all layouts transposed so contractions sit on partitions):
  qT = Wq @ h           (KV on partitions, S free)        f32r matmuls
  KT = Wk @ cond^T      (KV on partitions, N free)
  V  = cond @ Wv^T      (N on partitions, KV free, per-head 65-stride with
                         an appended ones column -> softmax denominators fall
                         out of the attention matmul for free)
  selector: centrality_i = phat_i . (sum_j phat_j)  (rank-equivalent to the
            reference's Smat row-sums); top-k mask via rank counting; mask
            fused into the exp() bias (per-partition, n on partitions)
  scoresT = KT_h^T-slices @ qT_h  (n on partitions, q free), exp on ScalarE
  attn@V with the ones column -> (65, q) PSUM, row 64 = denominator
  divide via reciprocal_approx + f32r ones-outer-product broadcast matmul,
  out-proj with +bo fused into the PSUM->SBUF copy on ScalarE
"""

import numpy as np

B, C, H, W = 8, 256, 32, 32
N = 512
COND = 512
KVD = 512
RD = 64
NH = 8
S = 1024
NEGB = 30000.0
N_CORES = 8

_cache = {}


def _build():
    import concourse.tile as tile
    import concourse.mybir as mybir
    from concourse import bacc
    import contextlib

    f32 = mybir.dt.float32
    f32r = mybir.dt.float32r
    A = mybir.AluOpType
    AF = mybir.ActivationFunctionType

    nc = bacc.Bacc("TRN2", target_bir_lowering=False, debug=False)

    bf16 = mybir.dt.bfloat16
    hS = nc.dram_tensor("hS", [C, S], bf16, kind="ExternalInput").ap()
    condT = nc.dram_tensor("condT", [COND, N], f32r, kind="ExternalInput").ap()
    maskc = nc.dram_tensor("maskc", [128, 4], f32, kind="ExternalInput").ap()
    WqT = nc.dram_tensor("WqT", [C, KVD], bf16, kind="ExternalInput").ap()
    WkT = nc.dram_tensor("WkT", [COND, KVD], bf16, kind="ExternalInput").ap()
    WvT = nc.dram_tensor("WvT", [COND, KVD], bf16, kind="ExternalInput").ap()
    WoT = nc.dram_tensor("WoT", [KVD, C], bf16, kind="ExternalInput").ap()
    WrkT = nc.dram_tensor("WrkT", [COND, RD], f32, kind="ExternalInput").ap()
    bqc = nc.dram_tensor("bqc", [128, 4], f32, kind="ExternalInput").ap()
    bkc = nc.dram_tensor("bkc", [128, 4], f32, kind="ExternalInput").ap()
    bvB = nc.dram_tensor("bvB", [128, KVD], f32, kind="ExternalInput").ap()
    brkB = nc.dram_tensor("brkB", [128, RD], f32, kind="ExternalInput").ap()
    boc = nc.dram_tensor("boc", [128, 2], f32, kind="ExternalInput").ap()
    onesc = nc.dram_tensor("onesc", [128, 1], f32, kind="ExternalInput").ap()
    onesr = nc.dram_tensor("onesr", [128, 1], bf16, kind="ExternalInput").ap()
    condTb = nc.dram_tensor("condTb", [COND, N], bf16, kind="ExternalInput").ap()
    onesw = nc.dram_tensor("onesw", [1, 64], f32r, kind="ExternalInput").ap()
    y = nc.dram_tensor("y", [C, S], f32, kind="ExternalOutput").ap()

    with tile.TileContext(nc) as tc, contextlib.ExitStack() as ctx:
        cons = ctx.enter_context(tc.tile_pool(name="cons", bufs=1))
        work = ctx.enter_context(tc.tile_pool(name="work", bufs=1))
        ppool = ctx.enter_context(tc.tile_pool(name="ppool", bufs=8))

        # ---------------- input DMAs (selector deps first) ----------------
        ct_t = [cons.tile([128, N], f32r, tag=f"ct{i}", name=f"ct{i}") for i in range(4)]
        for i in range(4):
            nc.sync.dma_start(ct_t[i][:], condT[128 * i:128 * (i + 1), :])
        wr_t = [cons.tile([128, RD], f32, tag=f"wr{i}", name=f"wr{i}") for i in range(4)]
        for i in range(4):
            nc.sync.dma_start(wr_t[i][:], WrkT[128 * i:128 * (i + 1), :])
        brkB_t = cons.tile([128, RD], f32, tag="brkB")
        nc.sync.dma_start(brkB_t[:], brkB)
        maskc_t = cons.tile([128, 4], f32, tag="maskc")
        nc.sync.dma_start(maskc_t[:], maskc)
        onesc_t = cons.tile([128, 1], f32, tag="onesc")
        nc.sync.dma_start(onesc_t[:], onesc)
        onesr_t = cons.tile([128, 1], bf16, tag="onesr")
        nc.sync.dma_start(onesr_t[:], onesr)
        onesw_t = cons.tile([1, 64], f32r, tag="onesw")
        nc.sync.dma_start(onesw_t[:], onesw)
        ctb_t = [cons.tile([128, N], bf16, tag=f"ctb{i}", name=f"ctb{i}") for i in range(4)]
        for i in range(4):
            nc.sync.dma_start(ctb_t[i][:], condTb[128 * i:128 * (i + 1), :])
        wk_t = [cons.tile([128, KVD], bf16, tag=f"wk{i}", name=f"wk{i}") for i in range(4)]
        wv_t = [cons.tile([128, KVD], bf16, tag=f"wv{i}", name=f"wv{i}") for i in range(4)]
        for i in range(4):
            nc.sync.dma_start(wk_t[i][:], WkT[128 * i:128 * (i + 1), :])
            nc.sync.dma_start(wv_t[i][:], WvT[128 * i:128 * (i + 1), :])
        wq_t = [cons.tile([128, KVD], bf16, tag=f"wq{i}", name=f"wq{i}") for i in range(2)]
        for i in range(2):
            nc.sync.dma_start(wq_t[i][:], WqT[128 * i:128 * (i + 1), :])
        h_t = [cons.tile([128, S], bf16, tag=f"h{i}", name=f"h{i}") for i in range(2)]
        for i in range(2):
            nc.sync.dma_start(h_t[i][:], hS[128 * i:128 * (i + 1), :])
        wo_t = [cons.tile([128, C], bf16, tag=f"wo{i}", name=f"wo{i}") for i in range(4)]
        for i in range(4):
            nc.sync.dma_start(wo_t[i][:], WoT[128 * i:128 * (i + 1), :])
        bqc_t = cons.tile([128, 4], f32, tag="bqc")
        nc.sync.dma_start(bqc_t[:], bqc)
        bkc_t = cons.tile([128, 4], f32, tag="bkc")
        nc.sync.dma_start(bkc_t[:], bkc)
        bvB_t = cons.tile([128, KVD], f32, tag="bvB")
        nc.sync.dma_start(bvB_t[:], bvB)
        boc_t = cons.tile([128, 2], f32, tag="boc")
        nc.sync.dma_start(boc_t[:], boc)

        with tc.tile_pool(name="psSel", bufs=2, space="PSUM") as psSel, \
             tc.tile_pool(name="psM", bufs=3, space="PSUM") as psM:
            # ---------------- selector (full fp32 path) ----------------
            c4 = work.tile([128, 4], f32, tag="c4")
            rank4 = work.tile([128, 4], f32, tag="rank4")
            biasb = work.tile([128, 4], f32, tag="biasb")
            ph_l = []
            for i in range(4):
                pp = psSel.tile([128, RD], f32, tag="psSel")
                for cc in range(4):
                    nc.tensor.matmul(pp[:], ct_t[cc][:, 128 * i:128 * (i + 1)].bitcast(f32),
                                     wr_t[cc][:], start=(cc == 0), stop=(cc == 3))
                Pn = work.tile([128, RD], f32, tag=f"Pn{i}", name=f"Pn{i}")
                nc.vector.tensor_tensor(Pn[:], pp[:], brkB_t[:], op=A.add)
                tmp64 = work.tile([128, RD], f32, tag=f"tmp64_{i}", name=f"tmp64_{i}")
                sq = work.tile([128, 1], f32, tag=f"sq{i}", name=f"sq{i}")
                nc.vector.tensor_tensor(tmp64[:], Pn[:], Pn[:], op=A.mult)
                nc.vector.reduce_sum(sq[:], tmp64[:], axis=mybir.AxisListType.X)
                lns = work.tile([128, 1], f32, tag=f"lns{i}", name=f"lns{i}")
                nc.scalar.activation(lns[:], sq[:], AF.Ln)
                rn = work.tile([128, 1], f32, tag=f"rn{i}", name=f"rn{i}")
                nc.scalar.activation(rn[:], lns[:], AF.Exp, scale=-0.5)
                ph = work.tile([128, RD], f32, tag=f"ph{i}", name=f"ph{i}")
                nc.vector.tensor_scalar(ph[:], Pn[:], rn[:, 0:1], None, op0=A.mult)
                ph_l.append(ph)
            sps = psSel.tile([1, RD], f32, tag="psSel")
            for i in range(4):
                nc.tensor.matmul(sps[:], onesc_t[:], ph_l[i][:], start=(i == 0), stop=(i == 3))
            s_row = work.tile([1, RD], f32, tag="s_row")
            nc.vector.tensor_copy(s_row[:], sps[:])
            sB = work.tile([128, RD], f32, tag="sB")
            nc.gpsimd.partition_broadcast(sB[:], s_row[:])
            for i in range(4):
                tmp64b = work.tile([128, RD], f32, tag=f"tmp64b_{i}", name=f"tmp64b_{i}")
                nc.vector.tensor_tensor(tmp64b[:], ph_l[i][:], sB[:], op=A.mult)
                nc.vector.reduce_sum(c4[:, i:i + 1], tmp64b[:], axis=mybir.AxisListType.X)
            cB = work.tile([128, N], f32, tag="cB")
            for j in range(4):
                crow = work.tile([1, 128], f32, tag=f"crow{j}", name=f"crow{j}")
                nc.sync.dma_start(crow[:], c4[:, j:j + 1])
                nc.gpsimd.partition_broadcast(cB[:, 128 * j:128 * (j + 1)], crow[:])
            cmpd = work.tile([128, N], f32, tag="cmpd")
            for i in range(4):
                nc.vector.tensor_scalar(cmpd[:], cB[:], c4[:, i:i + 1], 0.0,
                                        op0=A.is_gt, op1=A.add,
                                        accum_out=rank4[:, i:i + 1])
            selm = work.tile([128, 4], f32, tag="selm")
            nc.vector.tensor_scalar(selm[:], rank4[:], 306.5, None, op0=A.is_lt)
            allowed4 = work.tile([128, 4], f32, tag="allowed4")
            nc.vector.tensor_tensor(allowed4[:], selm[:], maskc_t[:], op=A.mult)
            nc.vector.tensor_scalar(biasb[:], allowed4[:], NEGB, NEGB,
                                    op0=A.mult, op1=A.subtract)

            # ---------------- projections (f32r) ----------------
            kt_t = [work.tile([128, N], bf16, tag=f"kt{i}", name=f"kt{i}") for i in range(4)]
            for kv in range(4):
                ps = psM.tile([128, N], f32, tag="psM")
                for cc in range(4):
                    nc.tensor.matmul(ps[:], wk_t[cc][:, 128 * kv:128 * (kv + 1)],
                                     ctb_t[cc][:], start=(cc == 0), stop=(cc == 3))
                nc.vector.tensor_scalar(kt_t[kv][:], ps[:], bkc_t[:, kv:kv + 1], None, op0=A.add)

            v520 = [work.tile([128, 520], bf16, tag=f"v520_{i}", name=f"v520_{i}") for i in range(4)]
            for nn_ in range(4):
                for hh in range(NH):
                    nc.vector.tensor_copy(v520[nn_][:, 65 * hh + 64:65 * hh + 65], onesr_t[:])
                ps = psM.tile([128, KVD], f32, tag="psM")
                for cc in range(4):
                    nc.tensor.matmul(ps[:], ctb_t[cc][:, 128 * nn_:128 * (nn_ + 1)],
                                     wv_t[cc][:], start=(cc == 0), stop=(cc == 3))
                vview = v520[nn_][:].rearrange("p (h c) -> p h c", c=65)[:, :, 0:64]
                nc.vector.tensor_tensor(vview,
                                        ps[:].rearrange("p (h c) -> p h c", c=64),
                                        bvB_t[:].rearrange("p (h c) -> p h c", c=64),
                                        op=A.add)

            qt_t = [work.tile([128, S], bf16, tag=f"qt{i}", name=f"qt{i}") for i in range(4)]
            for kv in range(4):
                for sc in range(2):
                    ps = psM.tile([128, 512], f32, tag="psM")
                    for cc in range(2):
                        nc.tensor.matmul(ps[:], wq_t[cc][:, 128 * kv:128 * (kv + 1)],
                                         h_t[cc][:, 512 * sc:512 * (sc + 1)],
                                         start=(cc == 0), stop=(cc == 1))
                    nc.vector.tensor_scalar(qt_t[kv][:, 512 * sc:512 * (sc + 1)], ps[:],
                                            bqc_t[:, kv:kv + 1], None, op0=A.add)

        # ---------------- attention ----------------
        with tc.tile_pool(name="psS", bufs=2, space="PSUM") as psS, \
             tc.tile_pool(name="psA", bufs=2, space="PSUM") as psA, \
             tc.tile_pool(name="psB", bufs=2, space="PSUM") as psB:
            att_t = [work.tile([128, S], bf16, tag=f"att{i}", name=f"att{i}") for i in range(4)]
            o65_l = {}
            for g in range(2):  # head groups (0-3, 4-7): denom/recip per group
                dstage = work.tile([8, 512], f32, tag=f"dstage{g}", name=f"dstage{g}")
                drecip = work.tile([8, 512], f32, tag=f"drecip{g}", name=f"drecip{g}")
                drecR = work.tile([8, 512], f32r, tag=f"drecR{g}", name=f"drecR{g}")
                dscr = work.tile([8, 512], f32, tag=f"dscr{g}", name=f"dscr{g}")
                for h in range(4 * g, 4 * g + 4):
                    i2 = h // 2
                    po = (h % 2) * 64
                    plist = []
                    for nn_ in range(4):
                        sps_ = psS.tile([128, 1024], f32, tag="psS")
                        for qc in range(2):
                            nc.tensor.matmul(sps_[:, 512 * qc:512 * (qc + 1)],
                                             kt_t[i2][po:po + 64, 128 * nn_:128 * (nn_ + 1)],
                                             qt_t[i2][po:po + 64, 512 * qc:512 * (qc + 1)],
                                             start=True, stop=True)
                        p_t = ppool.tile([128, 1024], bf16, tag="p", name=f"p_{h}_{nn_}")
                        nc.scalar.activation(p_t[:], sps_[:], AF.Exp,
                                             bias=biasb[:, nn_:nn_ + 1], scale=0.125)
                        plist.append(p_t)
                    augs = [psA.tile([65, 512], f32, tag="psA", name=f"aug_{h}_{qc}")
                            for qc in range(2)]
                    for nn_ in range(4):
                        for qc in range(2):
                            nc.tensor.matmul(augs[qc][:], v520[nn_][:, 65 * h:65 * h + 65],
                                             plist[nn_][:, 512 * qc:512 * (qc + 1)],
                                             start=(nn_ == 0), stop=(nn_ == 3))
                    for qc in range(2):
                        k = 2 * h + qc
                        o65 = work.tile([65, 512], f32, tag=f"o65_{k % 8}", name=f"o65_{k}")
                        nc.vector.tensor_copy(o65[:], augs[qc][:])
                        nc.sync.dma_start(dstage[k - 8 * g:k - 8 * g + 1, :], o65[64:65, :])
                        o65_l[k] = o65
                nc.vector.reciprocal_approx_accurate(drecip[:], dstage[:], dscr[:])
                nc.vector.tensor_copy(drecR[:], drecip[:])
                for h in range(4 * g, 4 * g + 4):
                    i2 = h // 2
                    po = (h % 2) * 64
                    for qc in range(2):
                        k = 2 * h + qc
                        rrow = work.tile([1, 512], f32r, tag=f"rrow{k}", name=f"rrow{k}")
                        nc.sync.dma_start(rrow[:], drecR[k - 8 * g:k - 8 * g + 1, :])
                        rps = psB.tile([64, 512], f32, tag="psB")
                        nc.tensor.matmul(rps[:], onesw_t[:], rrow[:],
                                         start=True, stop=True)
                        nc.vector.tensor_tensor(att_t[i2][po:po + 64, 512 * qc:512 * (qc + 1)],
                                                o65_l[k][0:64, :], rps[:], op=A.mult)

            # ---------------- output projection ----------------
            outF = [work.tile([128, S], f32, tag=f"outF{i}", name=f"outF{i}") for i in range(2)]
            for ccn in range(2):
                for sc in range(2):
                    ps = psB.tile([128, 512], f32, tag="psB")
                    for kv in range(4):
                        nc.tensor.matmul(ps[:], wo_t[kv][:, 128 * ccn:128 * (ccn + 1)],
                                         att_t[kv][:, 512 * sc:512 * (sc + 1)],
                                         start=(kv == 0), stop=(kv == 3))
                    nc.scalar.add(outF[ccn][:, 512 * sc:512 * (sc + 1)], ps[:],
                                  boc_t[:, ccn:ccn + 1])
            for ccn in range(2):
                nc.sync.dma_start(y[128 * ccn:128 * (ccn + 1), :], outF[ccn][:])

    nc.compile()
    return nc


def _get_nc():
    if "nc" not in _cache:
        _cache["nc"] = _build()
    return _cache["nc"]


def make_in_maps(**inputs):
    import ml_dtypes
    bf = ml_dtypes.bfloat16
    h = np.asarray(inputs["h"], np.float32)
    cond = np.asarray(inputs["cond_feats"], np.float32)
    cmask = np.asarray(inputs["cond_mask"])
    f = np.float32
    shared = {
        "WqT": np.ascontiguousarray(np.asarray(inputs["Wq"], f).T).astype(bf),
        "WkT": np.ascontiguousarray(np.asarray(inputs["Wk"], f).T).astype(bf),
        "WvT": np.ascontiguousarray(np.asarray(inputs["Wv"], f).T).astype(bf),
        "WoT": np.ascontiguousarray(np.asarray(inputs["Wo"], f).T).astype(bf),
        "WrkT": np.ascontiguousarray(np.asarray(inputs["Wrk"], f).T),
        "bqc": np.ascontiguousarray(np.asarray(inputs["bq"], f).reshape(4, 128).T),
        "bkc": np.ascontiguousarray(np.asarray(inputs["bk"], f).reshape(4, 128).T),
        "bvB": np.ascontiguousarray(np.broadcast_to(np.asarray(inputs["bv"], f), (128, KVD))),
        "brkB": np.ascontiguousarray(np.broadcast_to(np.asarray(inputs["brk"], f), (128, RD))),
        "boc": np.ascontiguousarray(np.asarray(inputs["bo"], f).reshape(2, 128).T),
        "onesc": np.ones((128, 1), f),
        "onesr": np.ones((128, 1), bf),
        "onesw": np.ones((1, 64), f),
    }
    in_maps = []
    for b in range(B):
        m = dict(shared)
        m["hS"] = np.ascontiguousarray(h[b].reshape(C, S)).astype(bf)
        m["condT"] = np.ascontiguousarray(cond[b].T)
        m["condTb"] = m["condT"].astype(bf)
        m["maskc"] = np.ascontiguousarray(cmask[b].astype(f).reshape(4, 128).T)
        in_maps.append(m)
    return in_maps


def kernel(**inputs):
    from concourse.bass_utils import run_bass_kernel_spmd
    nc = _get_nc()
    in_maps = make_in_maps(**inputs)
    res = run_bass_kernel_spmd(nc, in_maps, core_ids=list(range(N_CORES)))
    return np.stack([res.results[b]["y"].reshape(C, H, W) for b in range(B)])
